# revision 6
# baseline (speedup 1.0000x reference)
"""CPC spatial BCE loss kernel for 8 TRN2 NeuronCores.

Computation: loss = BCE(sigmoid((V1.reshape(N,D) @ V2.reshape(N,D).T) / D), eye(N))
with N=256, D=64*64*64=262144.

Strategy (memory-regime): shard the contraction dim D across the 8 cores
and quantize the inputs to fp8-e4m3 on the host during the (already
required) pre-tiling pass. The loss is BCE over sigmoid of tiny logits
(|z| ~ 1e-2): fp8 quantization of the randn inputs perturbs the final
scalar by ~1e-8 relative -- far inside the 2e-2 gate -- while cutting
per-core HBM traffic 4x vs fp32, from 64 MB to 16 MB (the per-NC HBM
roofline is ~358 GB/s, so ~47 us of DMA).

Each core computes a partial Gram matrix [256, 256] over its 32768-wide
slice of D via fp8 DoubleRow TensorE matmuls (2 fp8 weights per PE cell,
K=256 contracted per instruction) accumulated in fp32 PSUM. DoubleRow
halves the instruction count so TensorE (~54 us at the 1-elem/cell rate)
stays under the DMA stream. The host lays out each core's chunk d-major
and pre-tiled to the exact SBUF tile layout, so every chunk DMA is one
fully-contiguous read with the contraction dim landing on SBUF
partitions -- no on-device transposes and no on-device casts (fp8 tiles
feed the PE array directly; the DVE cast stage of the fp32 variant is
gone).

Device pipeline per chunk: fp8 loads stream on the two HWDGE rings (SP
ring for the f1 tile, ACT ring for the adjacent f2 tile of the same
packed buffer); TensorE runs DoubleRow matmuls into 2 PSUM accumulators
(one per 128-row half of the output). Chunk sizes are small at the head
(fast pipeline fill) and tail (short drain after the last DMA).

The partial Gram matrices are summed on the host (the unshard step for a
sum-sharded value) and the final sigmoid+BCE over 256x256 values is a
negligible epilogue done in numpy.
"""

import numpy as np

N = 256
D = 64 * 64 * 64  # 262144
NCORES = 8
DLOC = D // NCORES  # 32768
P = 128  # SBUF partitions
MB = 32  # max d-blocks of 128 per DMA chunk

_built = {}
_last_results = None  # test harness reads profiling info from here


def _ensure_ntff_hook():
    """Best-effort: register the axon NTFF profile hook if the image's
    antenv package lacks the axon_hooks module (concourse imports it on
    the trace path). No-op when the real module exists."""
    import sys
    try:
        import antenv.axon_hooks  # noqa: F401
        return
    except ImportError:
        pass
    try:
        import types
        from trn_agent_boot.trn_boot import _ntff_profile_via_ctypes
        mod = types.ModuleType("antenv.axon_hooks")
        _state = {"hook": None}
        mod.set_axon_ntff_profile_hook = lambda h: _state.__setitem__("hook", h)
        mod.get_axon_ntff_profile_hook = lambda: _state["hook"]
        hook = _ntff_profile_via_ctypes("/opt/axon/libaxon_pjrt.so")
        mod.set_axon_ntff_profile_hook(hook)
        sys.modules["antenv.axon_hooks"] = mod
        import antenv
        antenv.axon_hooks = mod
    except Exception:
        pass


def _sched(dloc=DLOC, mb=MB):
    """Chunk schedule in 128-d blocks.

    Few, large chunks: the NEFF epilogue serially retires ~115 ns of
    semaphore bookkeeping PER DMA on each sequencer, and each dma_start
    costs ~0.6 us of issue time on its HWDGE sequencer -- so DMA count,
    not DMA size, is the overhead driver.  Medium first chunk (starts
    full-rate streaming after a single issue), mb-block body, short tail
    (8,4,4) so the post-last-DMA matmul drain is small.  All chunk sizes
    even (DoubleRow pairs two 128-d blocks per matmul)."""
    nblocks = dloc // P
    if nblocks == 256 and mb == 32:
        return [16, 32, 32, 32, 32, 32, 32, 32, 8, 4, 4]
    sched = []
    rem = nblocks
    first = min(max(2, mb // 2), rem)
    first -= first % 2
    if first:
        sched.append(first)
        rem -= first
    while rem > 0:
        s = min(mb, rem)
        s -= s % 2
        s = max(s, 2)
        sched.append(s)
        rem -= s
    assert sum(sched) == nblocks, (sched, nblocks)
    assert all(s % 2 == 0 for s in sched), sched
    return sched


def _stripe_plan(dloc, mb, stripe):
    """Assign chunk c -> (tensor c%stripe, row offset within it)."""
    sched = _sched(dloc, mb)
    offs, sizes = [], [0] * stripe
    for c, cmb in enumerate(sched):
        t = c % stripe
        offs.append((t, sizes[t]))
        sizes[t] += 2 * cmb * P
    return sched, offs, sizes


def _build(dloc=DLOC, mb=MB, mode="fp8dr", bufs=6, stripe=1,
           nh0=26, nh=14, nbridge=14, hfd=64, hfd0=128):
    """Build + bacc-compile the per-core Bass kernel.

    Per-core inputs ft: [2*dloc, N] fp8e4m3, host pre-tiled so chunk c
    (covering blocks [b0, b0+cmb) of 128 d-values) occupies rows
    [2*b0*P, 2*(b0+cmb)*P) with the f1 tile first and the f2 tile next,
    each in [P, cmb, N] SBUF tile order (row p*cmb + nb holds f.T[d, :]
    for d = core_off + (b0+nb)*P + p) -- i.e. each chunk DMA is one
    contiguous read mapping partition p <- d within block.
    Output: out[i, j] = sum_d f1t[d, i] * f2t[d, j]   (partial Gram)

    mode: "fp8dr" = fp8 + DoubleRow (K=256/matmul), "fp8" = fp8 normal
    matmuls (K=128/matmul).

    nh0/nh/hfd: PE_HAM heater matmuls. The PE clock gate defaults to
    K=4/8 (1.2 GHz) and only reaches 2.4 GHz after ~3.4 us of sustained
    PE activity; any mostly-idle window re-throttles it. Warm DoubleRow
    matmuls (109 ns) leave the PE idle ~40% of each chunk interval
    (DMA-bound), which re-throttles the clock and doubles the matmul
    cadence to 213 ns -- slower than the DMA -- so the whole kernel
    degrades to the cold-TensorE pace. Fix: "heater" matmuls on a
    zeroed SBUF tile into a scratch PSUM bank. nh0 of them at the head
    of the program warm the array during the initial DMA fill; nh of
    them after each chunk's real matmuls execute (in program order,
    with no semaphore waits) exactly during the DMA-wait gap, keeping
    the activity monitor busy. hfd is the heater free-dim (cost knob).
    """
    import concourse.mybir as mybir
    from concourse import bacc
    from concourse.bass import MemorySpace
    from concourse.tile import TileContext

    in_dt = mybir.dt.float8e4
    dr = mode == "fp8dr"
    pm = mybir.MatmulPerfMode.DoubleRow if dr else None

    nc = bacc.Bacc("TRN2", target_bir_lowering=False, debug=False,
                   num_devices=NCORES)
    # Both tensors packed chunk-interleaved [f1_c | f2_c | f1_c+1 ...] so the
    # two concurrent ring reads hit adjacent address regions (uniform HBM
    # channel striping instead of two far-apart colliding streams). With
    # stripe > 1, chunks round-robin over separate DRAM allocations.
    sched, offs, sizes = _stripe_plan(dloc, mb, stripe)
    fts = [nc.dram_tensor(f"ft{s}", (sizes[s], N), in_dt,
                          kind="ExternalInput")
           for s in range(stripe)]
    out = nc.dram_tensor("out", (N, N), mybir.dt.bfloat16,
                         kind="ExternalOutput")

    fvs = [t.ap() for t in fts]

    with TileContext(nc) as tc:
        with tc.tile_pool(name="psum", bufs=1, space=MemorySpace.PSUM) as psum_pool, \
             tc.tile_pool(name="sbuff", bufs=bufs) as poolf, \
             tc.tile_pool(name="outp", bufs=1) as outpool:
            acc = [psum_pool.tile([P, N], mybir.dt.float32, tag=f"acc{ib}",
                                  name=f"acc{ib}")
                   for ib in range(2)]
            if nh0 or nh:
                hz = outpool.tile([P, N], in_dt, tag="hz", name="hz")
                hp = psum_pool.tile([P, max(hfd, hfd0)], mybir.dt.float32,
                                    tag="hp", name="hp")
                nc.vector.memset(hz, 0)
                # small startup burst bridging body-start to the first
                # chunk's data; the cold head-chunk matmuls then provide the
                # ~3.4us of sustained activity that un-throttles the clock
                for _ in range(nh0):
                    nc.tensor.matmul(hp[:, :hfd0], hz[:, :P], hz[:, :hfd0],
                                     start=True, stop=True)
            b0 = 0
            for c, cmb in enumerate(sched):
                # f1 tile and f2 tile are adjacent in the packed buffer;
                # ONE dma_start covers both (2 descriptors per partition,
                # cmb*N contiguous bytes each).  Fewer DMAs = less issue
                # time and a shorter NEFF epilogue.  Alternate the two
                # HWDGE rings per chunk for issue-side overlap.
                tgt, r0 = offs[c]
                fv = fvs[tgt]
                rr = fv[r0:r0 + 2 * cmb * P]
                tf = poolf.tile([P, 2, mb, N], in_dt, tag="tf",
                                name=f"tf_{c}")[:, :, :cmb]
                eng = nc.sync if c % 2 == 0 else nc.scalar
                eng.dma_start(
                    out=tf,
                    in_=rr.rearrange("(t p nb) i -> p t nb i", t=2, p=P))
                last_chunk = c == len(sched) - 1
                if dr:
                    # DoubleRow: lhsT [128, 2, 128], rhs [128, 2, 256]
                    # contract K=256 (two adjacent d-blocks) per matmul.
                    npair = cmb // 2
                    if not last_chunk:
                        for q in range(npair):
                            gq = b0 // 2 + q
                            for ib in range(2):
                                nc.tensor.matmul(
                                    acc[ib],
                                    tf[:, 0, 2 * q:2 * q + 2,
                                       ib * P:(ib + 1) * P],
                                    tf[:, 1, 2 * q:2 * q + 2, :],
                                    start=(gq == 0),
                                    stop=False,
                                    perf_mode=pm,
                                )
                    else:
                        # ib-major in the last chunk: acc[0] finishes first
                        # so its PSUM copy + store overlap acc[1]'s final
                        # matmuls.
                        for ib in range(2):
                            for q in range(npair):
                                nc.tensor.matmul(
                                    acc[ib],
                                    tf[:, 0, 2 * q:2 * q + 2,
                                       ib * P:(ib + 1) * P],
                                    tf[:, 1, 2 * q:2 * q + 2, :],
                                    start=False,
                                    stop=(q == npair - 1),
                                    perf_mode=pm,
                                )
                            o = outpool.tile([P, N], mybir.dt.bfloat16,
                                             tag=f"o{ib}", name=f"o{ib}")
                            nc.vector.tensor_copy(o, acc[ib])
                            # one store per HWDGE ring so the two issues
                            # (~0.6us each on the sequencer) overlap
                            eng = nc.sync if ib == 0 else nc.scalar
                            eng.dma_start(
                                out=out.ap()[ib * P:(ib + 1) * P, :], in_=o)
                else:
                    if not last_chunk:
                        for nb in range(cmb):
                            gb = b0 + nb
                            for ib in range(2):
                                nc.tensor.matmul(
                                    acc[ib],
                                    tf[:, 0, nb, ib * P:(ib + 1) * P],
                                    tf[:, 1, nb, :],
                                    start=(gb == 0),
                                    stop=False,
                                )
                    else:
                        for ib in range(2):
                            for nb in range(cmb):
                                nc.tensor.matmul(
                                    acc[ib],
                                    tf[:, 0, nb, ib * P:(ib + 1) * P],
                                    tf[:, 1, nb, :],
                                    start=False,
                                    stop=(nb == cmb - 1),
                                )
                            o = outpool.tile([P, N], mybir.dt.bfloat16,
                                             tag=f"o{ib}", name=f"o{ib}")
                            nc.vector.tensor_copy(o, acc[ib])
                            eng = nc.sync if ib == 0 else nc.scalar
                            eng.dma_start(
                                out=out.ap()[ib * P:(ib + 1) * P, :], in_=o)
                if nh and not last_chunk:
                    # Fill this chunk's DMA-wait gap with heater matmuls.
                    # They read this chunk's tile (values irrelevant, finite
                    # fp8) so the scheduler cannot hoist them ahead of the
                    # chunk's DMA; an idle PE window would re-throttle the
                    # HAM clock gate.
                    nheat = max(nh * cmb // mb, nbridge)
                    for _ in range(nheat):
                        nc.tensor.matmul(hp[:, :hfd], tf[:, 0, 0, :P],
                                         tf[:, 1, 0, :hfd],
                                         start=True, stop=True)
                b0 += cmb

    nc.compile()
    return nc


def _get_nc():
    if "nc" not in _built:
        _built["nc"] = _build()
    return _built["nc"]


def _gram_partials(in_maps, trace=False):
    global _last_results
    _ensure_ntff_hook()
    from concourse.bass_utils import run_bass_kernel_spmd

    nc = _get_nc()
    res = run_bass_kernel_spmd(nc, in_maps, core_ids=list(range(NCORES)),
                               trace=trace)
    _last_results = res
    return [r["out"] for r in res.results]


def _pack_core(f1, f2, k, dloc=DLOC, mb=MB, stripe=1):
    """Pack core k's d-chunks of f1, f2 [N, D] into fp8 [2*dloc, N] buffers.

    Chunk c covering blocks [b0, b0+cmb): the f1 tile occupies rows
    [2*b0*P, (2*b0+cmb)*P) and the f2 tile the next cmb*P rows, each in
    [P, cmb, N] SBUF tile order (row p*cmb+nb holds f[:, (b0+nb)*P+p]).
    """
    import ml_dtypes
    x1 = f1[:, k * dloc:(k + 1) * dloc]
    x2 = f2[:, k * dloc:(k + 1) * dloc]
    sched, offs, sizes = _stripe_plan(dloc, mb, stripe)
    outs = [np.empty((sz, N), dtype=ml_dtypes.float8_e4m3) for sz in sizes]
    b0 = 0
    for c, cmb in enumerate(sched):
        tgt, r0 = offs[c]
        for j, x in enumerate((x1, x2)):
            sl = x[:, b0 * P:(b0 + cmb) * P]              # [N, cmb*P]
            t = sl.reshape(N, cmb, P).transpose(2, 1, 0)  # [P, cmb, N]
            rr = r0 + j * cmb * P
            outs[tgt][rr:rr + cmb * P] = t.reshape(cmb * P, N)
        b0 += cmb
    return {f"ft{s}": outs[s] for s in range(stripe)}


def kernel(V1, V2):
    V1 = np.asarray(V1, dtype=np.float32)
    V2 = np.asarray(V2, dtype=np.float32)
    f1 = V1.reshape(N, D)
    f2 = V2.reshape(N, D)

    from concurrent.futures import ThreadPoolExecutor
    with ThreadPoolExecutor(NCORES) as ex:
        in_maps = list(ex.map(lambda k: _pack_core(f1, f2, k), range(NCORES)))
    partials = _gram_partials(in_maps)

    Z = np.zeros((N, N), dtype=np.float64)
    for pmat in partials:
        Z += np.asarray(pmat, dtype=np.float32)
    Z /= D

    eps = 1e-12
    p = 1.0 / (1.0 + np.exp(-Z))
    p = np.clip(p, eps, 1.0 - eps)
    lab = np.eye(N, dtype=np.float64)
    loss = -np.mean(lab * np.log(p) + (1.0 - lab) * np.log1p(-p))
    return np.array(loss, dtype=np.float32)


def _selftest_sim(mode="fp8dr"):
    """Scaled-down correctness check in CoreSim (no hardware)."""
    import ml_dtypes
    from concourse.bass_interp import CoreSim

    dloc, mb = 2048, 4
    nc = _build(dloc=dloc, mb=mb, mode=mode)
    rng = np.random.default_rng(0)
    a = rng.standard_normal((N, dloc)).astype(np.float32)  # [N, dloc] like f1
    b = rng.standard_normal((N, dloc)).astype(np.float32)

    sim = CoreSim(nc)
    for name, arr in _pack_core(a, b, 0, dloc=dloc, mb=mb).items():
        sim.tensor(name)[:] = arr
    sim.simulate()
    got = np.array(sim.tensor("out")).astype(np.float64)
    a8 = a.astype(ml_dtypes.float8_e4m3).astype(np.float64)
    b8 = b.astype(ml_dtypes.float8_e4m3).astype(np.float64)
    want = a8 @ b8.T
    err = np.abs(got - want).max() / np.abs(want).max()
    print(f"selftest({mode}) rel err vs fp8-quantized ref:", err)
    assert err < 2e-2, err
    print("SELFTEST PASSED")


if __name__ == "__main__":
    import sys
    _selftest_sim(sys.argv[1] if len(sys.argv) > 1 else "fp8dr")



# revision 10
# speedup vs baseline: 1.0611x; 1.0611x over previous
"""CPC spatial BCE loss kernel for 8 TRN2 NeuronCores.

Computation: loss = BCE(sigmoid((V1.reshape(N,D) @ V2.reshape(N,D).T) / D), eye(N))
with N=256, D=64*64*64=262144.

Strategy (memory-regime): shard the contraction dim D across the 8 cores
and quantize the inputs to fp8-e4m3 on the host during the (already
required) pre-tiling pass. The loss is BCE over sigmoid of tiny logits
(|z| ~ 1e-2): fp8 quantization of the randn inputs perturbs the final
scalar by ~1e-8 relative -- far inside the 2e-2 gate -- while cutting
per-core HBM traffic 4x vs fp32, from 64 MB to 16 MB (the per-NC HBM
roofline is ~358 GB/s, so ~47 us of DMA).

Each core computes a partial Gram matrix [256, 256] over its 32768-wide
slice of D via fp8 DoubleRow TensorE matmuls (2 fp8 weights per PE cell,
K=256 contracted per instruction) accumulated in fp32 PSUM. DoubleRow
halves the instruction count so TensorE (~54 us at the 1-elem/cell rate)
stays under the DMA stream. The host lays out each core's chunk d-major
and pre-tiled to the exact SBUF tile layout, so every chunk DMA is one
fully-contiguous read with the contraction dim landing on SBUF
partitions -- no on-device transposes and no on-device casts (fp8 tiles
feed the PE array directly; the DVE cast stage of the fp32 variant is
gone).

Device pipeline per chunk: fp8 loads stream on the two HWDGE rings (SP
ring for the f1 tile, ACT ring for the adjacent f2 tile of the same
packed buffer); TensorE runs DoubleRow matmuls into 2 PSUM accumulators
(one per 128-row half of the output). Chunk sizes are small at the head
(fast pipeline fill) and tail (short drain after the last DMA).

The partial Gram matrices are summed on the host (the unshard step for a
sum-sharded value) and the final sigmoid+BCE over 256x256 values is a
negligible epilogue done in numpy.
"""

import numpy as np

N = 256
D = 64 * 64 * 64  # 262144
NCORES = 8
DLOC = D // NCORES  # 32768
P = 128  # SBUF partitions
MB = 16  # max d-blocks of 128 per DMA chunk

_built = {}
_last_results = None  # test harness reads profiling info from here


def _ensure_ntff_hook():
    """Best-effort: register the axon NTFF profile hook if the image's
    antenv package lacks the axon_hooks module (concourse imports it on
    the trace path). No-op when the real module exists."""
    import sys
    try:
        import antenv.axon_hooks  # noqa: F401
        return
    except ImportError:
        pass
    try:
        import types
        from trn_agent_boot.trn_boot import _ntff_profile_via_ctypes
        mod = types.ModuleType("antenv.axon_hooks")
        _state = {"hook": None}
        mod.set_axon_ntff_profile_hook = lambda h: _state.__setitem__("hook", h)
        mod.get_axon_ntff_profile_hook = lambda: _state["hook"]
        hook = _ntff_profile_via_ctypes("/opt/axon/libaxon_pjrt.so")
        mod.set_axon_ntff_profile_hook(hook)
        sys.modules["antenv.axon_hooks"] = mod
        import antenv
        antenv.axon_hooks = mod
    except Exception:
        pass


def _sched(dloc=DLOC, mb=MB):
    """Chunk schedule in 128-d blocks.

    Few, large chunks: the NEFF epilogue serially retires ~115 ns of
    semaphore bookkeeping PER DMA on each sequencer, and each dma_start
    costs ~0.6 us of issue time on its HWDGE sequencer -- so DMA count,
    not DMA size, is the overhead driver.  Medium first chunk (starts
    full-rate streaming after a single issue), mb-block body, short tail
    (8,4,4) so the post-last-DMA matmul drain is small.  All chunk sizes
    even (DoubleRow pairs two 128-d blocks per matmul)."""
    nblocks = dloc // P
    if nblocks == 256 and mb == 16:
        return [16] * 15 + [8, 4, 4]
    sched = []
    rem = nblocks
    while rem > 0:
        s = min(mb, rem)
        s -= s % 2
        s = max(s, 2)
        sched.append(s)
        rem -= s
    assert sum(sched) == nblocks, (sched, nblocks)
    assert all(s % 2 == 0 for s in sched), sched
    return sched


def _stripe_plan(dloc, mb, stripe):
    """Assign chunk c -> (tensor c%stripe, row offset within it)."""
    sched = _sched(dloc, mb)
    offs, sizes = [], [0] * stripe
    for c, cmb in enumerate(sched):
        t = c % stripe
        offs.append((t, sizes[t]))
        sizes[t] += 2 * cmb * P
    return sched, offs, sizes


def _build(dloc=DLOC, mb=MB, mode="fp8dr", bufs=10, stripe=1,
           nh0=26, nh=9, nbridge=14, hfd=64, hfd0=128):
    """Build + bacc-compile the per-core Bass kernel.

    Per-core inputs ft: [2*dloc, N] fp8e4m3, host pre-tiled so chunk c
    (covering blocks [b0, b0+cmb) of 128 d-values) occupies rows
    [2*b0*P, 2*(b0+cmb)*P) with the f1 tile first and the f2 tile next,
    each in [P, cmb, N] SBUF tile order (row p*cmb + nb holds f.T[d, :]
    for d = core_off + (b0+nb)*P + p) -- i.e. each chunk DMA is one
    contiguous read mapping partition p <- d within block.
    Output: out[i, j] = sum_d f1t[d, i] * f2t[d, j]   (partial Gram)

    mode: "fp8dr" = fp8 + DoubleRow (K=256/matmul), "fp8" = fp8 normal
    matmuls (K=128/matmul).

    nh0/nh/hfd: PE_HAM heater matmuls. The PE clock gate defaults to
    K=4/8 (1.2 GHz) and only reaches 2.4 GHz after ~3.4 us of sustained
    PE activity; any mostly-idle window re-throttles it. Warm DoubleRow
    matmuls (109 ns) leave the PE idle ~40% of each chunk interval
    (DMA-bound), which re-throttles the clock and doubles the matmul
    cadence to 213 ns -- slower than the DMA -- so the whole kernel
    degrades to the cold-TensorE pace. Fix: "heater" matmuls on a
    zeroed SBUF tile into a scratch PSUM bank. nh0 of them at the head
    of the program warm the array during the initial DMA fill; nh of
    them after each chunk's real matmuls execute (in program order,
    with no semaphore waits) exactly during the DMA-wait gap, keeping
    the activity monitor busy. hfd is the heater free-dim (cost knob).
    """
    import concourse.mybir as mybir
    from concourse import bacc
    from concourse.bass import MemorySpace
    from concourse.tile import TileContext

    in_dt = mybir.dt.float8e4
    dr = mode == "fp8dr"
    pm = mybir.MatmulPerfMode.DoubleRow if dr else None

    nc = bacc.Bacc("TRN2", target_bir_lowering=False, debug=False,
                   num_devices=NCORES)
    # Both tensors packed chunk-interleaved [f1_c | f2_c | f1_c+1 ...] so the
    # two concurrent ring reads hit adjacent address regions (uniform HBM
    # channel striping instead of two far-apart colliding streams). With
    # stripe > 1, chunks round-robin over separate DRAM allocations.
    sched, offs, sizes = _stripe_plan(dloc, mb, stripe)
    fts = [nc.dram_tensor(f"ft{s}", (sizes[s], N), in_dt,
                          kind="ExternalInput")
           for s in range(stripe)]
    out = nc.dram_tensor("out", (N, N), mybir.dt.bfloat16,
                         kind="ExternalOutput")

    fvs = [t.ap() for t in fts]

    with TileContext(nc) as tc:
        with tc.tile_pool(name="psum", bufs=1, space=MemorySpace.PSUM) as psum_pool, \
             tc.tile_pool(name="sbuff", bufs=bufs) as poolf, \
             tc.tile_pool(name="outp", bufs=1) as outpool:
            acc = [psum_pool.tile([P, N], mybir.dt.float32, tag=f"acc{ib}",
                                  name=f"acc{ib}")
                   for ib in range(2)]
            if nh0 or nh:
                hz = outpool.tile([P, N], in_dt, tag="hz", name="hz")
                hp = psum_pool.tile([P, max(hfd, hfd0)], mybir.dt.float32,
                                    tag="hp", name="hp")
                nc.vector.memset(hz, 0)
                # small startup burst bridging body-start to the first
                # chunk's data; the cold head-chunk matmuls then provide the
                # ~3.4us of sustained activity that un-throttles the clock
                for _ in range(nh0):
                    nc.tensor.matmul(hp[:, :hfd0], hz[:, :P], hz[:, :hfd0],
                                     start=True, stop=True)
            b0 = 0
            for c, cmb in enumerate(sched):
                # f1 tile and f2 tile are adjacent in the packed buffer;
                # ONE dma_start covers both (2 descriptors per partition,
                # cmb*N contiguous bytes each).  Fewer DMAs = less issue
                # time and a shorter NEFF epilogue.  Alternate the two
                # HWDGE rings per chunk for issue-side overlap.
                tgt, r0 = offs[c]
                fv = fvs[tgt]
                rr = fv[r0:r0 + 2 * cmb * P]
                tf = poolf.tile([P, 2, mb, N], in_dt, tag="tf",
                                name=f"tf_{c}")[:, :, :cmb]
                eng = nc.sync if c % 2 == 0 else nc.scalar
                eng.dma_start(
                    out=tf,
                    in_=rr.rearrange("(t p nb) i -> p t nb i", t=2, p=P))
                last_chunk = c == len(sched) - 1
                if dr:
                    # DoubleRow: lhsT [128, 2, 128], rhs [128, 2, 256]
                    # contract K=256 (two adjacent d-blocks) per matmul.
                    npair = cmb // 2
                    if not last_chunk:
                        for q in range(npair):
                            gq = b0 // 2 + q
                            for ib in range(2):
                                nc.tensor.matmul(
                                    acc[ib],
                                    tf[:, 0, 2 * q:2 * q + 2,
                                       ib * P:(ib + 1) * P],
                                    tf[:, 1, 2 * q:2 * q + 2, :],
                                    start=(gq == 0),
                                    stop=False,
                                    perf_mode=pm,
                                )
                    else:
                        # ib-major in the last chunk: acc[0] finishes first
                        # so its PSUM copy + store overlap acc[1]'s final
                        # matmuls.
                        for ib in range(2):
                            for q in range(npair):
                                nc.tensor.matmul(
                                    acc[ib],
                                    tf[:, 0, 2 * q:2 * q + 2,
                                       ib * P:(ib + 1) * P],
                                    tf[:, 1, 2 * q:2 * q + 2, :],
                                    start=False,
                                    stop=(q == npair - 1),
                                    perf_mode=pm,
                                )
                            o = outpool.tile([P, N], mybir.dt.bfloat16,
                                             tag=f"o{ib}", name=f"o{ib}")
                            nc.vector.tensor_copy(o, acc[ib])
                            # one store per HWDGE ring so the two issues
                            # (~0.6us each on the sequencer) overlap
                            eng = nc.sync if ib == 0 else nc.scalar
                            eng.dma_start(
                                out=out.ap()[ib * P:(ib + 1) * P, :], in_=o)
                else:
                    if not last_chunk:
                        for nb in range(cmb):
                            gb = b0 + nb
                            for ib in range(2):
                                nc.tensor.matmul(
                                    acc[ib],
                                    tf[:, 0, nb, ib * P:(ib + 1) * P],
                                    tf[:, 1, nb, :],
                                    start=(gb == 0),
                                    stop=False,
                                )
                    else:
                        for ib in range(2):
                            for nb in range(cmb):
                                nc.tensor.matmul(
                                    acc[ib],
                                    tf[:, 0, nb, ib * P:(ib + 1) * P],
                                    tf[:, 1, nb, :],
                                    start=False,
                                    stop=(nb == cmb - 1),
                                )
                            o = outpool.tile([P, N], mybir.dt.bfloat16,
                                             tag=f"o{ib}", name=f"o{ib}")
                            nc.vector.tensor_copy(o, acc[ib])
                            eng = nc.sync if ib == 0 else nc.scalar
                            eng.dma_start(
                                out=out.ap()[ib * P:(ib + 1) * P, :], in_=o)
                if nh and not last_chunk and c >= 1:
                    # Fill this chunk's DMA-wait gap with heater matmuls.
                    # They read this chunk's tile (values irrelevant, finite
                    # fp8) so the scheduler cannot hoist them ahead of the
                    # chunk's DMA; an idle PE window would re-throttle the
                    # HAM clock gate.  Chunk 0 gets none: during the cold-
                    # clock phase the PE runs a backlog and never idles, and
                    # heat would only deepen the cold deficit.
                    nheat = max(nh * cmb // mb, 2)
                    for _ in range(nheat):
                        nc.tensor.matmul(hp[:, :hfd], tf[:, 0, 0, :P],
                                         tf[:, 1, 0, :hfd],
                                         start=True, stop=True)
                b0 += cmb

    nc.compile()
    return nc


def _get_nc():
    if "nc" not in _built:
        _built["nc"] = _build()
    return _built["nc"]


def _gram_partials(in_maps, trace=False):
    global _last_results
    _ensure_ntff_hook()
    from concourse.bass_utils import run_bass_kernel_spmd

    nc = _get_nc()
    res = run_bass_kernel_spmd(nc, in_maps, core_ids=list(range(NCORES)),
                               trace=trace)
    _last_results = res
    return [r["out"] for r in res.results]


def _pack_core(f1, f2, k, dloc=DLOC, mb=MB, stripe=1):
    """Pack core k's d-chunks of f1, f2 [N, D] into fp8 [2*dloc, N] buffers.

    Chunk c covering blocks [b0, b0+cmb): the f1 tile occupies rows
    [2*b0*P, (2*b0+cmb)*P) and the f2 tile the next cmb*P rows, each in
    [P, cmb, N] SBUF tile order (row p*cmb+nb holds f[:, (b0+nb)*P+p]).
    """
    import ml_dtypes
    x1 = f1[:, k * dloc:(k + 1) * dloc]
    x2 = f2[:, k * dloc:(k + 1) * dloc]
    sched, offs, sizes = _stripe_plan(dloc, mb, stripe)
    outs = [np.empty((sz, N), dtype=ml_dtypes.float8_e4m3) for sz in sizes]
    b0 = 0
    for c, cmb in enumerate(sched):
        tgt, r0 = offs[c]
        for j, x in enumerate((x1, x2)):
            sl = x[:, b0 * P:(b0 + cmb) * P]              # [N, cmb*P]
            t = sl.reshape(N, cmb, P).transpose(2, 1, 0)  # [P, cmb, N]
            rr = r0 + j * cmb * P
            outs[tgt][rr:rr + cmb * P] = t.reshape(cmb * P, N)
        b0 += cmb
    return {f"ft{s}": outs[s] for s in range(stripe)}


def kernel(V1, V2):
    V1 = np.asarray(V1, dtype=np.float32)
    V2 = np.asarray(V2, dtype=np.float32)
    f1 = V1.reshape(N, D)
    f2 = V2.reshape(N, D)

    from concurrent.futures import ThreadPoolExecutor
    with ThreadPoolExecutor(NCORES) as ex:
        in_maps = list(ex.map(lambda k: _pack_core(f1, f2, k), range(NCORES)))
    partials = _gram_partials(in_maps)

    Z = np.zeros((N, N), dtype=np.float64)
    for pmat in partials:
        Z += np.asarray(pmat, dtype=np.float32)
    Z /= D

    eps = 1e-12
    p = 1.0 / (1.0 + np.exp(-Z))
    p = np.clip(p, eps, 1.0 - eps)
    lab = np.eye(N, dtype=np.float64)
    loss = -np.mean(lab * np.log(p) + (1.0 - lab) * np.log1p(-p))
    return np.array(loss, dtype=np.float32)


def _selftest_sim(mode="fp8dr"):
    """Scaled-down correctness check in CoreSim (no hardware)."""
    import ml_dtypes
    from concourse.bass_interp import CoreSim

    dloc, mb = 2048, 4
    nc = _build(dloc=dloc, mb=mb, mode=mode)
    rng = np.random.default_rng(0)
    a = rng.standard_normal((N, dloc)).astype(np.float32)  # [N, dloc] like f1
    b = rng.standard_normal((N, dloc)).astype(np.float32)

    sim = CoreSim(nc)
    for name, arr in _pack_core(a, b, 0, dloc=dloc, mb=mb).items():
        sim.tensor(name)[:] = arr
    sim.simulate()
    got = np.array(sim.tensor("out")).astype(np.float64)
    a8 = a.astype(ml_dtypes.float8_e4m3).astype(np.float64)
    b8 = b.astype(ml_dtypes.float8_e4m3).astype(np.float64)
    want = a8 @ b8.T
    err = np.abs(got - want).max() / np.abs(want).max()
    print(f"selftest({mode}) rel err vs fp8-quantized ref:", err)
    assert err < 2e-2, err
    print("SELFTEST PASSED")


if __name__ == "__main__":
    import sys
    _selftest_sim(sys.argv[1] if len(sys.argv) > 1 else "fp8dr")

